# revision 4
# baseline (speedup 1.0000x reference)
"""MoE (top-2 of 8 experts, SwiGLU FFN) on 8 Trainium2 NeuronCores.

Strategy: expert-parallel. Routing (gate matmul + top-2 + softmax) is done
on the host in numpy; tokens are gathered per expert, padded to a common
capacity C, and each core runs the full SwiGLU FFN for one expert's tokens
with bf16 matmuls (fp32 PSUM accumulation). The host scatters the per-core
outputs back with the gate weights.

Device layouts (per core, pre-tiled on host so every DMA is contiguous):
  xt  [128, KD, C]  bf16   xT tiles: xt[p, k, c] = x_gathered[c, k*128+p]
  w0t [128, KD, H]  bf16   w0[e].T tiles (d on partitions, h on free)
  w1t [128, KD, H]  bf16
  w2t [128, KH, D]  bf16   w2[e].T tiles (h on partitions, d on free)
  b0t/b1t [128, KH] fp32   per-partition bias for the h0T/h1T layout
  out [C, D]        fp32

Stage 1 computes h0^T/h1^T (h on partitions, tokens on free dim) so that
stage 2 can contract over h without any on-device transpose.
"""

import math

import numpy as np
import ml_dtypes

E, TOPK, D, H = 8, 2, 1024, 2048
NCORES = 8
P = 128
KD = D // P   # 8 d-tiles
KH = H // P   # 16 h-tiles
BF16 = ml_dtypes.bfloat16

_build_cache: dict = {}


def _build_bass(C: int, repeat: int = 1):
    """Build the single-core SPMD Bass program for capacity C."""
    import concourse.bass as bass  # noqa: F401  (AP types used via tile)
    import concourse.bacc as bacc
    import concourse.mybir as mybir
    from concourse import tile

    fp32 = mybir.dt.float32
    bf16 = mybir.dt.bfloat16
    AF = mybir.ActivationFunctionType
    ALU = mybir.AluOpType

    # token chunks (free dim of stage-1 matmuls); each a multiple of 128, <=512
    chunks = []
    left = C
    while left > 0:
        w = min(512, left)
        chunks.append(w)
        left -= w

    nc = bacc.Bacc("TRN2", target_bir_lowering=False)
    xt_d = nc.dram_tensor("xt", [P, KD, C], bf16, kind="ExternalInput")
    w0_d = nc.dram_tensor("w0t", [P, KD, H], bf16, kind="ExternalInput")
    w1_d = nc.dram_tensor("w1t", [P, KD, H], bf16, kind="ExternalInput")
    w2_d = nc.dram_tensor("w2t", [P, KH, D], bf16, kind="ExternalInput")
    b0_d = nc.dram_tensor("b0t", [P, KH], fp32, kind="ExternalInput")
    b1_d = nc.dram_tensor("b1t", [P, KH], fp32, kind="ExternalInput")
    out_d = nc.dram_tensor("out", [C, D], fp32, kind="ExternalOutput")

    with tile.TileContext(nc) as tc:
        with (
            tc.tile_pool(name="wconst", bufs=1) as wpool,
            tc.tile_pool(name="act", bufs=2) as apool,
            tc.tile_pool(name="sil", bufs=4) as spool,
            tc.tile_pool(name="osb", bufs=2) as opool,
            tc.tile_pool(name="ps0", bufs=2, space="PSUM") as pp0,
            tc.tile_pool(name="ps1", bufs=2, space="PSUM") as pp1,
            tc.tile_pool(name="pso", bufs=2, space="PSUM") as ppo,
        ):
            w0_sb = wpool.tile([P, KD, H], bf16, tag="w0")
            w1_sb = wpool.tile([P, KD, H], bf16, tag="w1")
            w2_sb = wpool.tile([P, KH, D], bf16, tag="w2")
            xt_sb = wpool.tile([P, KD, C], bf16, tag="xt")
            b0_sb = wpool.tile([P, KH], fp32, tag="b0")
            b1_sb = wpool.tile([P, KH], fp32, tag="b1")
            nc.sync.dma_start(b0_sb[:], b0_d[:])
            nc.sync.dma_start(b1_sb[:], b1_d[:])
            nc.sync.dma_start(w1_sb[:], w1_d[:])
            nc.sync.dma_start(xt_sb[:], xt_d[:])
            nc.sync.dma_start(w0_sb[:], w0_d[:])
            nc.sync.dma_start(w2_sb[:], w2_d[:])

            for _ in range(repeat):
                c0 = 0
                for tcw in chunks:
                    act_sb = apool.tile([P, KH, tcw], bf16, tag="act")
                    for ht in range(KH):
                        hs = slice(ht * P, (ht + 1) * P)
                        ps1 = pp1.tile([P, tcw], fp32, tag="ps1")
                        for dk in range(KD):
                            nc.tensor.matmul(
                                ps1[:],
                                w1_sb[:, dk, hs],
                                xt_sb[:, dk, c0:c0 + tcw],
                                start=(dk == 0),
                                stop=(dk == KD - 1),
                            )
                        ps0 = pp0.tile([P, tcw], fp32, tag="ps0")
                        for dk in range(KD):
                            nc.tensor.matmul(
                                ps0[:],
                                w0_sb[:, dk, hs],
                                xt_sb[:, dk, c0:c0 + tcw],
                                start=(dk == 0),
                                stop=(dk == KD - 1),
                            )
                        sil = spool.tile([P, tcw], fp32, tag="sil")
                        nc.scalar.activation(
                            sil[:], ps1[:], AF.Silu, bias=b1_sb[:, ht:ht + 1]
                        )
                        # act = (h0 + b0) * silu(h1 + b1), cast to bf16 on write
                        nc.vector.scalar_tensor_tensor(
                            act_sb[:, ht, :],
                            ps0[:],
                            b0_sb[:, ht:ht + 1],
                            sil[:],
                            ALU.add,
                            ALU.mult,
                        )
                    for cs in range(tcw // P):
                        o_sb = opool.tile([P, D], fp32, tag="osb")
                        for dh in range(D // 512):
                            pso = ppo.tile([P, 512], fp32, tag="pso")
                            for ht in range(KH):
                                nc.tensor.matmul(
                                    pso[:],
                                    act_sb[:, ht, cs * P:(cs + 1) * P],
                                    w2_sb[:, ht, dh * 512:(dh + 1) * 512],
                                    start=(ht == 0),
                                    stop=(ht == KH - 1),
                                )
                            nc.scalar.activation(
                                o_sb[:, dh * 512:(dh + 1) * 512], pso[:], AF.Copy
                            )
                        nc.sync.dma_start(
                            out_d[c0 + cs * P:c0 + (cs + 1) * P, :], o_sb[:]
                        )
                    c0 += tcw
    nc.compile()
    return nc


def _get_bass(C: int, repeat: int = 1):
    key = (C, repeat)
    if key not in _build_cache:
        _build_cache[key] = _build_bass(C, repeat)
    return _build_cache[key]


def _route(x2d: np.ndarray, gate_w: np.ndarray, gate_b: np.ndarray):
    """Top-2 routing on the host (f64 logits for stable ordering)."""
    lg = x2d.astype(np.float64) @ gate_w.astype(np.float64).T
    lg += gate_b.astype(np.float64)
    order = np.argsort(-lg, axis=1, kind="stable")
    ti = order[:, :TOPK]
    tv = np.take_along_axis(lg, ti, axis=1)
    m = tv.max(axis=1, keepdims=True)
    ew = np.exp(tv - m)
    wk = ew / ew.sum(axis=1, keepdims=True)
    return ti, wk


def _tile_kxm(a: np.ndarray, ktiles: int) -> np.ndarray:
    """[Kdim, M] -> [128, ktiles, M] with Kdim = ktiles*128 on partitions."""
    kdim, m = a.shape
    assert kdim == ktiles * P
    return np.ascontiguousarray(a.reshape(ktiles, P, m).transpose(1, 0, 2))


def kernel(x, gate_w, gate_b, w0, b0, w1, b1, w2, b2):
    from concourse.bass_utils import run_bass_kernel_spmd

    x = np.asarray(x)
    gate_w = np.asarray(gate_w, dtype=np.float32)
    gate_b = np.asarray(gate_b, dtype=np.float32)
    w0 = np.asarray(w0, dtype=np.float32)
    b0 = np.asarray(b0, dtype=np.float32)
    w1 = np.asarray(w1, dtype=np.float32)
    b1 = np.asarray(b1, dtype=np.float32)
    w2 = np.asarray(w2, dtype=np.float32)
    b2 = np.asarray(b2, dtype=np.float32)

    Bn, Sq, Dv = x.shape
    T = Bn * Sq
    x2d = np.ascontiguousarray(x.reshape(T, Dv)).astype(np.float32, copy=False)

    ti, wk = _route(x2d, gate_w, gate_b)

    idxs, wgts = [], []
    for e in range(E):
        sel = [np.nonzero(ti[:, k] == e)[0] for k in range(TOPK)]
        idxs.append(np.concatenate(sel))
        wgts.append(np.concatenate([wk[s, k] for k, s in enumerate(sel)]))

    maxc = max(len(i) for i in idxs)
    C = max(P, int(math.ceil(maxc / P)) * P)
    nc = _get_bass(C)

    x2d_bf = x2d.astype(BF16)
    in_maps = []
    for e in range(E):
        xg = np.zeros((C, Dv), dtype=BF16)
        xg[: len(idxs[e])] = x2d_bf[idxs[e]]
        in_maps.append(
            {
                "xt": _tile_kxm(np.ascontiguousarray(xg.T), KD),
                "w0t": _tile_kxm(np.ascontiguousarray(w0[e].T.astype(BF16)), KD),
                "w1t": _tile_kxm(np.ascontiguousarray(w1[e].T.astype(BF16)), KD),
                "w2t": _tile_kxm(np.ascontiguousarray(w2[e].T.astype(BF16)), KH),
                "b0t": np.ascontiguousarray(b0[e].reshape(KH, P).T),
                "b1t": np.ascontiguousarray(b1[e].reshape(KH, P).T),
            }
        )

    res = run_bass_kernel_spmd(nc, in_maps, core_ids=list(range(NCORES)))

    out = np.zeros((T, Dv), dtype=np.float32)
    for e in range(E):
        n = len(idxs[e])
        o = np.asarray(res.results[e]["out"][:n], dtype=np.float32)
        out[idxs[e]] += wgts[e][:, None].astype(np.float32) * (o + b2[e][None, :])
    return out.reshape(Bn, Sq, Dv)


# revision 24
# speedup vs baseline: 1.2069x; 1.2069x over previous
"""MoE (top-2 of 8 experts, SwiGLU FFN) on 8 Trainium2 NeuronCores.

Strategy: expert-parallel. Routing (gate matmul + top-2 + softmax) is done
on the host in numpy; tokens are gathered per expert, padded to a common
capacity C, and each core runs the full SwiGLU FFN for one expert's tokens
with bf16 matmuls (fp32 PSUM accumulation). The host scatters the per-core
outputs back with the gate weights.

Device layouts (per core, pre-tiled on host so every DMA is contiguous):
  xt  [128, KD, C]  bf16   xT tiles: xt[p, k, c] = x_gathered[c, k*128+p]
  w0t [128, KD, H]  bf16   w0[e].T tiles (d on partitions, h on free)
  w1t [128, KD, H]  bf16
  w2t [128, KH, D]  bf16   w2[e].T tiles (h on partitions, d on free)
  b0t/b1t [128, KH] fp32   per-partition bias for the h0T/h1T layout
  out [C, D]        fp32

Stage 1 computes h0^T/h1^T (h on partitions, tokens on free dim) so that
stage 2 can contract over h without any on-device transpose.
"""

import math

import numpy as np
import ml_dtypes

E, TOPK, D, H = 8, 2, 1024, 2048
NCORES = 8
P = 128
KD = D // P   # 8 d-tiles
KH = H // P   # 16 h-tiles
BF16 = ml_dtypes.bfloat16

_build_cache: dict = {}


def _chunk_plan(C: int):
    """Token-chunk widths: remainder first (PE warms up while DMA streams),
    512s in the middle, 256 last (shorter drain tail)."""
    if C <= 512:
        return [C]
    rem = C - 256
    n512 = rem // 512
    head = rem - n512 * 512
    plan = ([head] if head else []) + [512] * n512 + [256]
    if head and head < 192 and n512 >= 1:
        a = head + 512
        plan = [a // 2, a - a // 2] + [512] * (n512 - 1) + [256]
    return plan


def _build_bass(C: int, repeat: int = 1):
    """Build the single-core SPMD Bass program for capacity C."""
    import concourse.bass as bass  # noqa: F401  (AP types used via tile)
    import concourse.bacc as bacc
    import concourse.mybir as mybir
    from concourse import tile

    fp32 = mybir.dt.float32
    bf16 = mybir.dt.bfloat16
    AF = mybir.ActivationFunctionType
    ALU = mybir.AluOpType

    # token chunks (free dim of the matmuls); any width <= 512. A smaller
    # first chunk lets the PE start while the bulk DMA is still in flight.
    chunks = _chunk_plan(C)

    nc = bacc.Bacc("TRN2", target_bir_lowering=False)
    xt_d = nc.dram_tensor("xt", [P, KD, C], bf16, kind="ExternalInput")
    w0_d = nc.dram_tensor("w0t", [P, KD, H], bf16, kind="ExternalInput")
    w1_d = nc.dram_tensor("w1t", [P, KD, H], bf16, kind="ExternalInput")
    w2_d = nc.dram_tensor("w2t", [P, KH, D], bf16, kind="ExternalInput")
    b0_d = nc.dram_tensor("b0t", [P, KH], fp32, kind="ExternalInput")
    b1_d = nc.dram_tensor("b1t", [P, KH], fp32, kind="ExternalInput")
    # out is produced transposed: out_t[p, k, c] = ffn_out[c, k*128+p]
    out_d = nc.dram_tensor("out", [P, KD, C], fp32, kind="ExternalOutput")

    with tile.TileContext(nc) as tc:
        with (
            tc.tile_pool(name="wconst", bufs=1) as wpool,
            tc.tile_pool(name="act", bufs=2) as apool,
            tc.tile_pool(name="sil", bufs=4) as spool,
            tc.tile_pool(name="osb", bufs=2) as opool,
            tc.tile_pool(name="ps0", bufs=2, space="PSUM") as pp0,
            tc.tile_pool(name="ps1", bufs=2, space="PSUM") as pp1,
            tc.tile_pool(name="pso", bufs=2, space="PSUM") as ppo,
        ):
            w0_sb = wpool.tile([P, KD, H], bf16, tag="w0")
            w1_sb = wpool.tile([P, KD, H], bf16, tag="w1")
            w2_sb = wpool.tile([P, KH, D], bf16, tag="w2")
            xt_sb = wpool.tile([P, KD, C], bf16, tag="xt")
            b0_sb = wpool.tile([P, KH], fp32, tag="b0")
            b1_sb = wpool.tile([P, KH], fp32, tag="b1")
            # Load order matters: pieces are drained in issue order on the
            # queue, so front-load exactly what the first matmuls need.
            hpieces = [(0, 128), (128, 128), (256, 256), (512, 512),
                       (1024, 512), (1536, 512)]
            h0_, hw_ = hpieces[0]
            nc.sync.dma_start(w1_sb[:, :, h0_:h0_ + hw_], w1_d[:, :, h0_:h0_ + hw_])
            nc.sync.dma_start(xt_sb[:, :, 0:chunks[0]], xt_d[:, :, 0:chunks[0]])
            nc.sync.dma_start(w0_sb[:, :, h0_:h0_ + hw_], w0_d[:, :, h0_:h0_ + hw_])
            nc.sync.dma_start(b0_sb[:], b0_d[:])
            nc.sync.dma_start(b1_sb[:], b1_d[:])
            for h0_, hw_ in hpieces[1:]:
                hs_ = slice(h0_, h0_ + hw_)
                nc.sync.dma_start(w1_sb[:, :, hs_], w1_d[:, :, hs_])
                nc.sync.dma_start(w0_sb[:, :, hs_], w0_d[:, :, hs_])
            cpos = chunks[0]
            for tcw_ in chunks[1:]:
                nc.sync.dma_start(
                    xt_sb[:, :, cpos:cpos + tcw_], xt_d[:, :, cpos:cpos + tcw_]
                )
                cpos += tcw_
            nc.sync.dma_start(w2_sb[:, :, 0:512], w2_d[:, :, 0:512])
            nc.sync.dma_start(w2_sb[:, :, 512:D], w2_d[:, :, 512:D])

            for _ in range(repeat):
                c0 = 0
                for tcw in chunks:
                    act_sb = apool.tile([P, KH, tcw], bf16, tag="act")
                    for ht in range(KH):
                        hs = slice(ht * P, (ht + 1) * P)
                        ps1 = pp1.tile([P, tcw], fp32, tag="ps1")
                        for dk in range(KD):
                            nc.tensor.matmul(
                                ps1[:],
                                w1_sb[:, dk, hs],
                                xt_sb[:, dk, c0:c0 + tcw],
                                start=(dk == 0),
                                stop=(dk == KD - 1),
                            )
                        ps0 = pp0.tile([P, tcw], fp32, tag="ps0")
                        for dk in range(KD):
                            nc.tensor.matmul(
                                ps0[:],
                                w0_sb[:, dk, hs],
                                xt_sb[:, dk, c0:c0 + tcw],
                                start=(dk == 0),
                                stop=(dk == KD - 1),
                            )
                        sil = spool.tile([P, tcw], fp32, tag="sil")
                        nc.scalar.activation(
                            sil[:], ps1[:], AF.Silu, bias=b1_sb[:, ht:ht + 1]
                        )
                        # act = (h0 + b0) * silu(h1 + b1), cast to bf16 on write
                        nc.vector.scalar_tensor_tensor(
                            act_sb[:, ht, :],
                            ps0[:],
                            b0_sb[:, ht:ht + 1],
                            sil[:],
                            ALU.add,
                            ALU.mult,
                        )
                    # stage 2 transposed: out_t[d-tile] = w2T_tile.T @ act
                    for dk in range(KD):
                        pso = ppo.tile([P, tcw], fp32, tag="pso")
                        for ht in range(KH):
                            nc.tensor.matmul(
                                pso[:],
                                w2_sb[:, ht, dk * P:(dk + 1) * P],
                                act_sb[:, ht, :],
                                start=(ht == 0),
                                stop=(ht == KH - 1),
                            )
                        o_sb = opool.tile([P, tcw], fp32, tag="osb")
                        nc.scalar.activation(o_sb[:], pso[:], AF.Copy)
                        nc.sync.dma_start(out_d[:, dk, c0:c0 + tcw], o_sb[:])
                    c0 += tcw
    nc.compile()
    return nc


def _get_bass(C: int, repeat: int = 1):
    key = (C, repeat)
    if key not in _build_cache:
        _build_cache[key] = _build_bass(C, repeat)
    return _build_cache[key]


_runner_cache: dict = {}


def _get_runner(C: int, repeat: int = 1):
    """Compile the SPMD program once and return a reusable launcher.

    Mirrors concourse.bass2jax.run_bass_via_pjrt but memoizes the jitted
    executable so repeated kernel() calls don't recompile the NEFF.
    """
    key = (C, repeat)
    if key in _runner_cache:
        return _runner_cache[key]

    import jax
    from jax.experimental.shard_map import shard_map
    from jax.sharding import Mesh, PartitionSpec
    import concourse.mybir as mybir
    from concourse import bass2jax

    nc = _get_bass(C, repeat)
    bass2jax.install_neuronx_cc_hook()
    partition_name = nc.partition_id_tensor.name if nc.partition_id_tensor else None

    in_names: list = []
    out_names: list = []
    out_avals: list = []
    out_shapes: list = []
    for alloc in nc.m.functions[0].allocations:
        if not isinstance(alloc, mybir.MemoryLocationSet):
            continue
        name = alloc.memorylocations[0].name
        if alloc.kind == "ExternalInput":
            if name != partition_name:
                in_names.append(name)
        elif alloc.kind == "ExternalOutput":
            shape = tuple(alloc.tensor_shape)
            dtype = mybir.dt.np(alloc.dtype)
            out_names.append(name)
            out_avals.append(jax.core.ShapedArray(shape, dtype))
            out_shapes.append((shape, dtype))
    n_params = len(in_names)
    all_names = list(in_names) + list(out_names)
    if partition_name is not None:
        all_names.append(partition_name)
    donate = tuple(range(n_params, n_params + len(out_names)))

    def _body(*args):
        operands = list(args)
        if partition_name is not None:
            operands.append(bass2jax.partition_id_tensor())
        outs = bass2jax._bass_exec_p.bind(
            *operands,
            out_avals=tuple(out_avals),
            in_names=tuple(all_names),
            out_names=tuple(out_names),
            lowering_input_output_aliases=(),
            sim_require_finite=True,
            sim_require_nnan=True,
            nc=nc,
        )
        return tuple(outs)

    devices = jax.devices()[:NCORES]
    assert len(devices) == NCORES
    mesh = Mesh(np.asarray(devices), ("core",))
    in_specs = (PartitionSpec("core"),) * (n_params + len(out_names))
    out_specs = (PartitionSpec("core"),) * len(out_names)
    sharded = jax.jit(
        shard_map(
            _body, mesh=mesh, in_specs=in_specs, out_specs=out_specs, check_rep=False
        ),
        donate_argnums=donate,
        keep_unused=True,
    )

    def run(in_maps):
        concat_in = [
            np.concatenate([np.asarray(in_maps[c][nm]) for c in range(NCORES)], axis=0)
            for nm in in_names
        ]
        concat_zeros = [
            np.zeros((NCORES * s[0], *s[1:]), dt) for s, dt in out_shapes
        ]
        out_arrs = sharded(*concat_in, *concat_zeros)
        return [
            {
                nm: np.asarray(out_arrs[i]).reshape(NCORES, *out_shapes[i][0])[c]
                for i, nm in enumerate(out_names)
            }
            for c in range(NCORES)
        ]

    _runner_cache[key] = run
    return run


def _route(x2d: np.ndarray, gate_w: np.ndarray, gate_b: np.ndarray):
    """Top-2 routing on the host (f64 logits for stable ordering)."""
    lg = x2d.astype(np.float64) @ gate_w.astype(np.float64).T
    lg += gate_b.astype(np.float64)
    order = np.argsort(-lg, axis=1, kind="stable")
    ti = order[:, :TOPK]
    tv = np.take_along_axis(lg, ti, axis=1)
    m = tv.max(axis=1, keepdims=True)
    ew = np.exp(tv - m)
    wk = ew / ew.sum(axis=1, keepdims=True)
    return ti, wk


def _tile_kxm(a: np.ndarray, ktiles: int) -> np.ndarray:
    """[Kdim, M] -> [128, ktiles, M] with Kdim = ktiles*128 on partitions."""
    kdim, m = a.shape
    assert kdim == ktiles * P
    return np.ascontiguousarray(a.reshape(ktiles, P, m).transpose(1, 0, 2))


def _prepare(x, gate_w, gate_b, w0, b0, w1, b1, w2, b2):
    """Host-side routing + per-core input packing. Returns (in_maps, meta)."""
    x = np.asarray(x)
    gate_w = np.asarray(gate_w, dtype=np.float32)
    gate_b = np.asarray(gate_b, dtype=np.float32)
    w0 = np.asarray(w0, dtype=np.float32)
    b0 = np.asarray(b0, dtype=np.float32)
    w1 = np.asarray(w1, dtype=np.float32)
    b1 = np.asarray(b1, dtype=np.float32)
    w2 = np.asarray(w2, dtype=np.float32)
    b2 = np.asarray(b2, dtype=np.float32)

    Bn, Sq, Dv = x.shape
    T = Bn * Sq
    x2d = np.ascontiguousarray(x.reshape(T, Dv)).astype(np.float32, copy=False)

    ti, wk = _route(x2d, gate_w, gate_b)

    idxs, wgts = [], []
    for e in range(E):
        sel = [np.nonzero(ti[:, k] == e)[0] for k in range(TOPK)]
        idxs.append(np.concatenate(sel))
        wgts.append(np.concatenate([wk[s, k] for k, s in enumerate(sel)]))

    maxc = max(len(i) for i in idxs)
    C = max(P, maxc)

    x2d_bf = x2d.astype(BF16)
    in_maps = []
    for e in range(E):
        xg = np.zeros((C, Dv), dtype=BF16)
        xg[: len(idxs[e])] = x2d_bf[idxs[e]]
        in_maps.append(
            {
                "xt": _tile_kxm(np.ascontiguousarray(xg.T), KD),
                "w0t": _tile_kxm(np.ascontiguousarray(w0[e].T.astype(BF16)), KD),
                "w1t": _tile_kxm(np.ascontiguousarray(w1[e].T.astype(BF16)), KD),
                "w2t": _tile_kxm(np.ascontiguousarray(w2[e].T.astype(BF16)), KH),
                "b0t": np.ascontiguousarray(b0[e].reshape(KH, P).T),
                "b1t": np.ascontiguousarray(b1[e].reshape(KH, P).T),
            }
        )
    meta = (Bn, Sq, Dv, T, C, idxs, wgts, b2)
    return in_maps, meta


def _combine(results, meta):
    Bn, Sq, Dv, T, C, idxs, wgts, b2 = meta
    out = np.zeros((T, Dv), dtype=np.float32)
    for e in range(E):
        n = len(idxs[e])
        # out_t [128, KD, C] -> [C, D] with d = k*128 + p
        ot = np.asarray(results[e]["out"])
        o = ot.transpose(2, 1, 0).reshape(C, Dv)[:n]
        out[idxs[e]] += wgts[e][:, None].astype(np.float32) * (o + b2[e][None, :])
    return out.reshape(Bn, Sq, Dv)


def kernel(x, gate_w, gate_b, w0, b0, w1, b1, w2, b2):
    in_maps, meta = _prepare(x, gate_w, gate_b, w0, b0, w1, b1, w2, b2)
    C = meta[4]
    run = _get_runner(C)
    results = run(in_maps)
    return _combine(results, meta)
